# revision 2
# baseline (speedup 1.0000x reference)
"""Trainium2 Bass kernel for nn_ConcatenatedIrrepsTensorProduct, v2.

Data-parallel over E=200000 edges on 8 cores (25088 padded edges each).

Everything is bf16 on the wire and feature-major on chip:
the host pre-transposes inputs to [features, edges] with a feature order
chosen so every tensor-product block lands in a dense K-stack, and
un-transposes the output.  No PE transposes, no PSUM evacuation copies
for transposition.

Per 512-edge tile, 14 matmul passes (N=512 each):
  p1, p2                MLP layers 1-2 (K=64)
  Ybc1g, Ybc2g, Ybc3g   y-broadcast stacks via K=4 selector matmuls
  BW1', BW2', BW3'      wt-permutation stacks from a2 (K=64)
  pGa, pG2, pG3         main tensor-product blocks (K=128/128/64)
  FA, F2, F3            folded (Wl@Wf) output contraction
Elementwise (DVE): Y_i = pG_i * Ybc_i ; Ms_i = Y_i * BW_i'.
Scalar engine: silu(p1), silu(p2), f32->bf16 output copy.
"""

import numpy as np

import concourse.bacc as bacc
import concourse.bass as bass
import concourse.mybir as mybir
import concourse.tile as tile
from concourse.bass_utils import run_bass_kernel_spmd

# ----------------------------------------------------------------------------
E = 200000
NCORES = 8
EC = E // NCORES              # 25000
TILE = 512
NT = (EC + TILE - 1) // TILE  # 49
ECP = NT * TILE               # 25088
GROUP = 4                     # tiles per DMA group (2048 edges)

MUL = 32
U = 64
SCALAR_DIM = 64
HID = 64
PW = 0.125
INV_S3 = 1.0 / np.sqrt(3.0)

F32 = mybir.dt.float32
BF16 = mybir.dt.bfloat16
NPBF16 = mybir.dt.np(BF16)

_CACHE = {}


def _silu_cst() -> float:
    z = np.linspace(-12.0, 12.0, 200001)
    phi = np.exp(-0.5 * z**2) / np.sqrt(2.0 * np.pi)
    s = z / (1.0 + np.exp(-z))
    trapz = getattr(np, "trapezoid", None) or np.trapz
    return float(1.0 / np.sqrt(trapz(s**2 * phi, z)))


# ----------------------------------------------------------------------------
# host-side constant folding


def build_consts(w0, w1, w2, w3, Wl0, Wl1, Wm1, Wm2, Wm3, Wf0, Wf1):
    f8 = 1.0 / np.sqrt(np.float64(U))        # 1/8
    fm = 1.0 / np.sqrt(np.float64(MUL))      # 1/sqrt(32)
    C = _silu_cst()

    w0p = (PW * w0).astype(np.float64)                 # [64,32]
    w1p = (PW * INV_S3 * w1).astype(np.float64)
    w2p = (PW * w2).astype(np.float64)
    w3p = (PW * w3).astype(np.float64)
    Wc0 = (Wl0.astype(np.float64) @ Wf0.astype(np.float64)) * (f8 * fm)  # [64,32]
    Wc1 = (Wl1.astype(np.float64) @ Wf1.astype(np.float64)) * (f8 * fm)

    Wm1s = Wm1.astype(np.float64) / np.sqrt(np.float64(SCALAR_DIM))      # [64,64]
    Wm2s = C * Wm2.astype(np.float64) / np.sqrt(np.float64(HID))         # [64,64]
    Wm3s = C * Wm3.astype(np.float64) / np.sqrt(np.float64(HID))         # [64,128]

    # --- G-stage lhsT -----------------------------------------------------
    # xT1 rows (K): [s0a; s0b; s1a0; s1b0], xT2 rows: [s1a1; s1b1; s1a2; s1b2]
    # pGa cols (M): [g_mid0; g_t3_0; g_t1_0; g_mid2]
    L1 = np.zeros((128, 128))
    L1[0:32, 0:32] = w0p[0:32]          # s0a -> mid0
    L1[32:64, 0:32] = w0p[32:64]        # s0b -> mid0
    L1[64:96, 32:64] = w3p[0:32]        # s1a0 -> t3_0
    L1[96:128, 32:64] = w3p[32:64]
    L1[64:96, 64:96] = w1p[0:32]        # s1a0 -> t1_0
    L1[96:128, 64:96] = w1p[32:64]
    L1[0:32, 96:128] = w2p[0:32]        # s0a -> mid2
    L1[32:64, 96:128] = w2p[32:64]

    # pG2 cols: [g_t3_1; g_t3_2; g_t1_1; g_t1_2]
    L2 = np.zeros((128, 128))
    L2[0:32, 0:32] = w3p[0:32]          # s1a1 -> t3_1
    L2[32:64, 0:32] = w3p[32:64]
    L2[64:96, 32:64] = w3p[0:32]        # s1a2 -> t3_2
    L2[96:128, 32:64] = w3p[32:64]
    L2[0:32, 64:96] = w1p[0:32]         # s1a1 -> t1_1
    L2[32:64, 64:96] = w1p[32:64]
    L2[64:96, 96:128] = w1p[0:32]       # s1a2 -> t1_2
    L2[96:128, 96:128] = w1p[32:64]

    # pG3 cols: [g_mid2; g_mid2]  (K = xT1 rows 0:64)
    L3 = np.zeros((64, 64))
    L3[0:32, 0:32] = w2p[0:32]
    L3[32:64, 0:32] = w2p[32:64]
    L3[0:32, 32:64] = w2p[0:32]
    L3[32:64, 32:64] = w2p[32:64]

    # --- BW' lhsT (wt permutation stacks from a2, K=64) -------------------
    # BW1' cols: [wt0:32; wt96:128; wt32:64; wt64:96]  (matches pGa rows)
    B1m = np.zeros((64, 128))
    B1m[:, 0:32] = Wm3s[:, 0:32]
    B1m[:, 32:64] = Wm3s[:, 96:128]
    B1m[:, 64:96] = Wm3s[:, 32:64]
    B1m[:, 96:128] = Wm3s[:, 64:96]
    # BW2' cols: [wt96; wt96; wt32; wt32]  (matches pG2 rows)
    B2m = np.zeros((64, 128))
    B2m[:, 0:32] = Wm3s[:, 96:128]
    B2m[:, 32:64] = Wm3s[:, 96:128]
    B2m[:, 64:96] = Wm3s[:, 32:64]
    B2m[:, 96:128] = Wm3s[:, 32:64]
    # BW3' cols: [wt64; wt64]  (matches pG3 rows)
    B3m = np.zeros((64, 64))
    B3m[:, 0:32] = Wm3s[:, 64:96]
    B3m[:, 32:64] = Wm3s[:, 64:96]

    # --- selector matmuls (K=4 y-rows [y0;y10;y11;y12]) -------------------
    # Ybc1g: [y0;y0;y10;y10] per 32 rows (matches pGa scale pattern)
    S1 = np.zeros((4, 128))
    S1[0, 0:64] = 1.0
    S1[1, 64:128] = 1.0
    # Ybc2g: [y0;y0;y11;y12] (matches pG2)
    S2 = np.zeros((4, 128))
    S2[0, 0:64] = 1.0
    S2[2, 64:96] = 1.0
    S2[3, 96:128] = 1.0
    # Ybc3g: [y11;y12] (matches pG3)
    S3 = np.zeros((4, 64))
    S3[2, 0:32] = 1.0
    S3[3, 32:64] = 1.0

    # --- F-stage lhsT: out rows m = [o0; o1k0; o1k1; o1k2] ----------------
    FA = np.zeros((128, 128))
    FA[0:32, 0:32] = Wc0[0:32]          # mid0_s -> o0
    FA[32:64, 32:64] = Wc1[32:64]       # t3_0_s -> o1k0
    FA[64:96, 0:32] = Wc0[32:64]        # t1_0_s -> o0 (mid1 contribution)
    FA[96:128, 32:64] = Wc1[0:32]       # mid2_s(k0) -> o1k0
    F2 = np.zeros((128, 128))
    F2[0:32, 64:96] = Wc1[32:64]        # t3_1_s -> o1k1
    F2[32:64, 96:128] = Wc1[32:64]      # t3_2_s -> o1k2
    F2[64:96, 0:32] = Wc0[32:64]        # t1_1_s -> o0
    F2[96:128, 0:32] = Wc0[32:64]       # t1_2_s -> o0
    F3 = np.zeros((64, 128))
    F3[0:32, 64:96] = Wc1[0:32]         # mid2_s(k1) -> o1k1
    F3[32:64, 96:128] = Wc1[0:32]       # mid2_s(k2) -> o1k2

    return {
        "Wm1s": Wm1s, "Wm2s": Wm2s,
        "L1": L1, "L2": L2, "L3": L3,
        "B1m": B1m, "B2m": B2m, "B3m": B3m,
        "S1": S1, "S2": S2, "S3": S3,
        "FA": FA, "F2": F2, "F3": F3,
    }


# const blob: [128, CB_COLS] bf16, every const packed column-wise
CB_LAYOUT = {}


def _mk_layout():
    off = 0
    for n, p, w in (
        ("Wm1s", 64, 64), ("Wm2s", 64, 64),
        ("L1", 128, 128), ("L2", 128, 128), ("L3", 64, 64),
        ("B1m", 64, 128), ("B2m", 64, 128), ("B3m", 64, 64),
        ("S1", 4, 128), ("S2", 4, 128), ("S3", 4, 64),
        ("FA", 128, 128), ("F2", 128, 128), ("F3", 64, 128),
    ):
        CB_LAYOUT[n] = (p, off, w)
        off += w
    return off


CB_COLS = _mk_layout()


def pack_consts(consts):
    cb = np.zeros((128, CB_COLS), dtype=NPBF16)
    for n, (p, c0, w) in CB_LAYOUT.items():
        cb[0:p, c0:c0 + w] = consts[n].astype(NPBF16)
    return cb


# output row m of the device -> column of the reference output
def _out_colmap():
    cols = np.zeros(128, dtype=np.int64)
    cols[0:32] = np.arange(32)
    for k in range(3):
        for w in range(32):
            cols[32 + 32 * k + w] = 32 + 3 * w + k
    return cols


OUT_COLS = _out_colmap()


# ----------------------------------------------------------------------------
# host data prep


def build_xc(x1a, x1b):
    """[E,128] x1a/x1b -> xc [256, E] bf16 with the K-stack feature order."""
    n = x1a.shape[0]
    s1a = x1a[:, 32:].reshape(n, 32, 3)
    s1b = x1b[:, 32:].reshape(n, 32, 3)
    xc = np.empty((256, n), dtype=NPBF16)
    xc[0:32] = x1a[:, :32].T
    xc[32:64] = x1b[:, :32].T
    xc[64:96] = s1a[:, :, 0].T
    xc[96:128] = s1b[:, :, 0].T
    xc[128:160] = s1a[:, :, 1].T
    xc[160:192] = s1b[:, :, 1].T
    xc[192:224] = s1a[:, :, 2].T
    xc[224:256] = s1b[:, :, 2].T
    return xc


# ----------------------------------------------------------------------------
# numpy emulation of the device dataflow (for fast correctness checks)


def numpy_sim(inputs):
    consts = build_consts(
        inputs["w0"], inputs["w1"], inputs["w2"], inputs["w3"],
        inputs["Wl0"], inputs["Wl1"],
        inputs["Wm1"], inputs["Wm2"], inputs["Wm3"],
        inputs["Wf0"], inputs["Wf1"],
    )
    f = np.float32

    def silu(x):
        return x / (1.0 + np.exp(-x))

    bf = lambda a: a.astype(NPBF16).astype(f)

    xc = build_xc(np.asarray(inputs["x1a"]), np.asarray(inputs["x1b"])).astype(f)
    xT1 = xc[0:128]
    xT2 = xc[128:256]
    scT = bf(np.asarray(inputs["scalars"]).T)
    yT = bf(np.asarray(inputs["x2"]).T)          # [4, E] = [y0;y10;y11;y12]

    c = {k: bf(v) for k, v in consts.items()}

    p1 = c["Wm1s"].T @ scT
    a1 = bf(silu(p1))
    p2 = c["Wm2s"].T @ a1
    a2 = bf(silu(p2))

    Ybc1 = c["S1"].T @ yT
    Ybc2 = c["S2"].T @ yT
    Ybc3 = c["S3"].T @ yT

    BW1 = c["B1m"].T @ a2
    BW2 = c["B2m"].T @ a2
    BW3 = c["B3m"].T @ a2

    pGa = c["L1"].T @ xT1
    pG2 = c["L2"].T @ xT2
    pG3 = c["L3"].T @ xT1[0:64]

    MsA = bf(bf(pGa * Ybc1) * BW1)
    Ms2 = bf(bf(pG2 * Ybc2) * BW2)
    Ms3 = bf(bf(pG3 * Ybc3) * BW3)

    pOF = c["FA"].T @ MsA + c["F2"].T @ Ms2 + c["F3"].T @ Ms3   # [128, E]

    n = xT1.shape[1]
    out = np.empty((n, 128), dtype=f)
    out[:, OUT_COLS] = pOF.T
    return out


# ----------------------------------------------------------------------------
# device kernel


def build_nc():
    nc = bacc.Bacc("TRN2", target_bir_lowering=False)

    xc_d = nc.declare_dram_parameter("xc", [256, ECP], BF16, isOutput=False)
    sc_d = nc.declare_dram_parameter("sc", [64, ECP], BF16, isOutput=False)
    y_d = nc.declare_dram_parameter("y", [4, ECP], BF16, isOutput=False)
    cb_d = nc.declare_dram_parameter("CB", [128, CB_COLS], BF16, isOutput=False)
    out_d = nc.declare_dram_parameter("out", [128, ECP], BF16, isOutput=True)

    SILU = mybir.ActivationFunctionType.Silu
    GE = GROUP * TILE  # 2048

    with tile.TileContext(nc) as tc:
        with (
            tc.tile_pool(name="consts", bufs=1) as cpool,
            tc.tile_pool(name="xin", bufs=2) as xpool,
            tc.tile_pool(name="mid", bufs=2) as mpool,
            tc.tile_pool(name="outp", bufs=2) as opool,
            tc.tile_pool(name="ps", bufs=1, space="PSUM") as ps,
        ):
            cb = cpool.tile([128, CB_COLS], BF16, tag="cb", name="cb")
            nc.sync.dma_start(cb[:], cb_d[:])
            csb = {n: cb[0:p, c0:c0 + w] for n, (p, c0, w) in CB_LAYOUT.items()}

            NG = (NT + GROUP - 1) // GROUP
            for g in range(NG):
                e0 = g * GE
                ge = min(GE, ECP - e0)
                ntl = ge // TILE

                xT1g = xpool.tile([128, GE], BF16, tag="xT1")
                nc.sync.dma_start(xT1g[:, :ge], xc_d[0:128, e0:e0 + ge])
                xT2g = xpool.tile([128, GE], BF16, tag="xT2")
                nc.sync.dma_start(xT2g[:, :ge], xc_d[128:256, e0:e0 + ge])
                scg = xpool.tile([64, GE], BF16, tag="scg")
                nc.sync.dma_start(scg[:, :ge], sc_d[:, e0:e0 + ge])
                yg = xpool.tile([4, GE], BF16, tag="yg")
                nc.sync.dma_start(yg[:, :ge], y_d[:, e0:e0 + ge])
                OFTg = opool.tile([128, GE], BF16, tag="OFT")

                for t in range(ntl):
                    s = slice(t * TILE, (t + 1) * TILE)
                    xT1 = xT1g[:, s]
                    xT2 = xT2g[:, s]
                    sct = scg[:, s]
                    yt = yg[:, s]

                    # ---- MLP (bank A: p1 rows 0:64, p2 rows 64:128) -----
                    pA = ps.tile([128, TILE], F32, tag="A", name="pA")
                    nc.tensor.matmul(pA[0:64, :], csb["Wm1s"], sct,
                                     start=True, stop=True,
                                     skip_group_check=True)
                    a1 = mpool.tile([64, TILE], BF16, tag="a1")
                    nc.scalar.activation(a1[:], pA[0:64, :], SILU)

                    # ---- selector broadcasts ----------------------------
                    pY1 = ps.tile([128, TILE], F32, tag="B", name="pY1")
                    nc.tensor.matmul(pY1[:], csb["S1"], yt, start=True, stop=True)
                    pY2 = ps.tile([128, TILE], F32, tag="C", name="pY2")
                    nc.tensor.matmul(pY2[:], csb["S2"], yt, start=True, stop=True)
                    pY3 = ps.tile([64, TILE], F32, tag="D", name="pY3")
                    nc.tensor.matmul(pY3[:], csb["S3"], yt, start=True, stop=True)

                    # ---- G stage ----------------------------------------
                    pGa = ps.tile([128, TILE], F32, tag="E", name="pGa")
                    nc.tensor.matmul(pGa[:], csb["L1"], xT1, start=True, stop=True)

                    # p2 into bank A rows 64:128
                    nc.tensor.matmul(pA[0:64, :], csb["Wm2s"], a1[:],
                                     start=True, stop=True,
                                     skip_group_check=True)
                    a2 = mpool.tile([64, TILE], BF16, tag="a2")
                    nc.scalar.activation(a2[:], pA[0:64, :], SILU)

                    pG2 = ps.tile([128, TILE], F32, tag="F", name="pG2")
                    nc.tensor.matmul(pG2[:], csb["L2"], xT2, start=True, stop=True)
                    pG3 = ps.tile([64, TILE], F32, tag="G", name="pG3")
                    nc.tensor.matmul(pG3[:], csb["L3"], xT1[0:64, :],
                                     start=True, stop=True)

                    # evacuate y-broadcast stacks to SBUF (walrus rejects
                    # tensor_tensor with both operands in PSUM)
                    yb1 = mpool.tile([128, TILE], BF16, tag="yb1")
                    nc.scalar.copy(yb1[:], pY1[:])
                    yb2 = mpool.tile([128, TILE], BF16, tag="yb2")
                    nc.scalar.copy(yb2[:], pY2[:])
                    yb3 = mpool.tile([64, TILE], BF16, tag="yb3")
                    nc.scalar.copy(yb3[:], pY3[:])
                    Y1 = mpool.tile([128, TILE], BF16, tag="Y1")
                    nc.vector.tensor_mul(Y1[:], pGa[:], yb1[:])
                    Y2 = mpool.tile([128, TILE], BF16, tag="Y2")
                    nc.vector.tensor_mul(Y2[:], pG2[:], yb2[:])
                    Y3 = mpool.tile([64, TILE], BF16, tag="Y3")
                    nc.vector.tensor_mul(Y3[:], pG3[:], yb3[:])

                    # ---- BW' stacks (K=64 from a2) ----------------------
                    pB1 = ps.tile([128, TILE], F32, tag="B", name="pB1")
                    nc.tensor.matmul(pB1[:], csb["B1m"], a2[:], start=True, stop=True)
                    pB2 = ps.tile([128, TILE], F32, tag="C", name="pB2")
                    nc.tensor.matmul(pB2[:], csb["B2m"], a2[:], start=True, stop=True)
                    pB3 = ps.tile([64, TILE], F32, tag="D", name="pB3")
                    nc.tensor.matmul(pB3[:], csb["B3m"], a2[:], start=True, stop=True)

                    MsA = mpool.tile([128, TILE], BF16, tag="MsA")
                    nc.vector.tensor_mul(MsA[:], pB1[:], Y1[:])
                    Ms2 = mpool.tile([128, TILE], BF16, tag="Ms2")
                    nc.vector.tensor_mul(Ms2[:], pB2[:], Y2[:])
                    Ms3 = mpool.tile([64, TILE], BF16, tag="Ms3")
                    nc.vector.tensor_mul(Ms3[:], pB3[:], Y3[:])

                    # ---- F stage ----------------------------------------
                    pOF = ps.tile([128, TILE], F32, tag="H", name="pOF")
                    nc.tensor.matmul(pOF[:], csb["FA"], MsA[:],
                                     start=True, stop=False)
                    nc.tensor.matmul(pOF[:], csb["F2"], Ms2[:],
                                     start=False, stop=False)
                    nc.tensor.matmul(pOF[:], csb["F3"], Ms3[:],
                                     start=False, stop=True)
                    nc.scalar.copy(OFTg[:, s], pOF[:])

                nc.sync.dma_start(out_d[:, e0:e0 + ge], OFTg[:, :ge])

    nc.finalize()
    return nc


def _get_nc():
    if "nc" not in _CACHE:
        _CACHE["nc"] = build_nc()
    return _CACHE["nc"]


# ----------------------------------------------------------------------------
# host entry point


def build_in_maps(inputs):
    inputs = {k: np.asarray(v) for k, v in inputs.items()}
    consts = build_consts(
        inputs["w0"], inputs["w1"], inputs["w2"], inputs["w3"],
        inputs["Wl0"], inputs["Wl1"],
        inputs["Wm1"], inputs["Wm2"], inputs["Wm3"],
        inputs["Wf0"], inputs["Wf1"],
    )
    cb = pack_consts(consts)
    xc = build_xc(inputs["x1a"], inputs["x1b"])        # [256, E] bf16
    scT = inputs["scalars"].T.astype(NPBF16)           # [64, E]
    yT = inputs["x2"].T.astype(NPBF16)                 # [4, E]

    pad = ECP - EC
    in_maps = []
    for c in range(NCORES):
        s = slice(c * EC, (c + 1) * EC)
        m = {
            "xc": np.pad(xc[:, s], ((0, 0), (0, pad))),
            "sc": np.pad(scT[:, s], ((0, 0), (0, pad))),
            "y": np.pad(yT[:, s], ((0, 0), (0, pad))),
            "CB": cb,
        }
        in_maps.append(m)
    return in_maps


def unpack_out(res_outs):
    """res_outs: list of per-core 'out' arrays [128, ECP] bf16."""
    full = np.empty((E, 128), dtype=np.float32)
    for c in range(NCORES):
        o = np.asarray(res_outs[c])[:, :EC].astype(np.float32).T  # [EC,128]
        full[c * EC:(c + 1) * EC][:, OUT_COLS] = o
    return full


def run(inputs, trace=False):
    in_maps = build_in_maps(inputs)
    nc = _get_nc()
    res = run_bass_kernel_spmd(nc, in_maps, list(range(NCORES)), trace=trace)
    out = unpack_out([res.results[c]["out"] for c in range(NCORES)])
    return out, res


def kernel(**inputs) -> np.ndarray:
    out, _ = run(inputs, trace=False)
    return out
